# revision 42
# baseline (speedup 1.0000x reference)
"""DenseNibblePPR diffusion kernel for 8 Trainium2 NeuronCores.

Math: out = ppr[idx] @ (X @ W + b),  shapes:
  X [16384, 128] f32, ppr [16384, 16384] f32, W [128, 64] f32,
  b [64] f32, idx [4096] i64  ->  out [4096, 64] f32.

Sharding strategy (batch / seed-node parallel):
  The 4096 seed nodes are split across the 8 cores (512 each). Each
  core receives its 512 gathered PPR rows, pre-transposed to
  [16384, 512] so the contraction dim (nodes) lands on SBUF
  partitions, plus the full [16384, 64] encoder table enc = X @ W + b
  (the encoder is 3% of the FLOPs; it is evaluated once during input
  sharding rather than redundantly per core). Each core streams its
  33.5 MB row shard from HBM in 1 MiB grouped DMAs and accumulates
  outT[64, 512] over 128 k-chunks in a single PSUM fp32 accumulation
  chain on the tensor engine. The host concatenates the per-core
  [512, 64] results. No collectives.

  GEMM precision (mm="bf16pair", default): fp32 operands are split
  into bf16 hi+lo pairs (packed hi|lo along the free dim so the DMA
  shape matches the fp32 layout exactly — same total bytes). The
  diffusion matmul only needs 64 stationary columns, so [enc_hi |
  enc_lo] loads as one [128, 128] stationary and each k-chunk takes
  just 2 matmul passes (rows_hi, rows_lo): PSUM partitions 0:64
  accumulate the enc_hi products and 64:128 the enc_lo products
  (including the lo*lo term), summed once by DVE at the end. This is
  bf16x4-grade fp32 emulation: measured end-to-end error 3.9e-6 (vs
  5.9e-7 for native fp32 matmuls) with the PE at 1 cycle/row instead
  of fp32's 4, which moves the kernel from PE-bound (114 us/core) to
  the measured HBM roofline (~101-104 us/core for the 37.9 MB/core
  stream, ~370 GB/s/core).

  Alternatives kept behind flags, all verified correct on HW:
    mm="fp32"  native fp32 matmuls, err 5.9e-7, ~114 us (PE-bound)
    mm="f32r"  TRN2 reduced-precision fp32 mode, err 1.5e-4, ~103 us
    encoder="replicated"/"allgather": on-device encoder variants;
    slower (131 us / ~180 us) — redundant encoder work or the
    AllGather sit on the PE critical path.
"""

import numpy as np

N = 16384
D_IN = 128
D_H = 64
B = 4096
N_CORES = 8
B_LOC = B // N_CORES  # 512
KC = N // 128  # 128 contraction chunks of 128 nodes
N_SH = N // N_CORES  # 2048 encoder shard rows per core
KC_SH = N_SH // 128  # 16 encoder chunks per core

_compiled_nc = None
_compiled_mode = None
_last_in_maps = None


def _build(reps=1, encoder="host", mm="fp32", dma_g=4, rows_bufs=8, main_f32r=None):
    import concourse.bacc as bacc
    import concourse.bass as bass
    import concourse.mybir as mybir
    import concourse.tile as tile

    if main_f32r:  # legacy alias
        mm = "f32r"
    f32 = mybir.dt.float32
    f32r = mybir.dt.float32r
    bf16 = mybir.dt.bfloat16
    main_f32r = mm == "f32r"
    pair = mm == "bf16pair"
    assert not (pair and encoder != "host"), "bf16pair requires host encoder"
    mm_dt = {"fp32": f32, "f32r": f32r, "bf16pair": bf16}[mm]

    nc = bacc.Bacc("TRN2", target_bir_lowering=False, debug=False, num_devices=N_CORES)

    if pair:
        # hi|lo planes packed along the free dim: row n = [hi(512|64), lo(...)]
        rows_pair = nc.dram_tensor("rows_pair", [N, 2 * B_LOC], bf16, kind="ExternalInput")
        enc_pair = nc.dram_tensor("enc_pair", [N, 2 * D_H], bf16, kind="ExternalInput")
    elif encoder == "host":
        rowsT = nc.dram_tensor("rowsT", [N, B_LOC], f32, kind="ExternalInput")
        enc_in = nc.dram_tensor("enc", [N, D_H], f32, kind="ExternalInput")
    else:
        rowsT = nc.dram_tensor("rowsT", [N, B_LOC], f32, kind="ExternalInput")
        xt_cols = N if encoder == "replicated" else N_SH
        xt = nc.dram_tensor("xt", [D_IN, xt_cols], f32, kind="ExternalInput")
        w = nc.dram_tensor("w", [D_IN, D_H], f32, kind="ExternalInput")
        bias = nc.dram_tensor("bias", [128, D_H], f32, kind="ExternalInput")
    outT = nc.dram_tensor("outT", [D_H, B_LOC], f32, kind="ExternalOutput")

    with tile.TileContext(nc) as tc:
        with (
            tc.tile_pool(name="const", bufs=1) as cpool,
            tc.tile_pool(name="enc", bufs=2 if encoder == "replicated" else 1) as encpool,
            tc.tile_pool(name="rows", bufs=rows_bufs) as rpool,
            tc.tile_pool(name="res", bufs=2) as opool,
            tc.tile_pool(name="psenc", bufs=4, space="PSUM") as psenc,
            tc.tile_pool(name="psout", bufs=2, space="PSUM") as psout,
            tc.tile_pool(name="dram", bufs=1, space="DRAM") as dram,
        ):
            for _rep in range(reps):
                # ---- encoder table: enc[n, h], n on partitions, 128 chunks
                # stored as 16 SBUF tiles [128, 8*64] (8 chunks each)
                def load_enc_tiles(src_handle, dtype, tagp, src_offset=0, bitcast=None):
                    tiles = []
                    for j in range(16):
                        t = encpool.tile([128, 8 * D_H], dtype, tag=f"{tagp}{j}")
                        src = bass.AP(
                            src_handle,
                            src_offset + j * 1024 * D_H,
                            [[D_H, 128], [128 * D_H, 8], [1, D_H]],
                        )
                        if bitcast is not None:
                            src = src.bitcast(bitcast)
                        nc.sync.dma_start(
                            t[:].rearrange("p (g h) -> p g h", g=8), src
                        )
                        tiles.append(t)
                    return lambda k: tiles[k // 8][
                        :, (k % 8) * D_H : (k % 8 + 1) * D_H
                    ]

                if pair:
                    ep_tiles = []
                    for j in range(16):
                        t = encpool.tile([128, 8 * 2 * D_H], bf16, tag=f"enc{j}")
                        src = bass.AP(
                            enc_pair,
                            j * 1024 * 2 * D_H,
                            [[2 * D_H, 128], [128 * 2 * D_H, 8], [1, 2 * D_H]],
                        )
                        nc.sync.dma_start(
                            t[:].rearrange("p (g h) -> p g h", g=8), src
                        )
                        ep_tiles.append(t)

                    # [enc_hi | enc_lo] as one [128, 128] stationary: one
                    # matmul pass produces both products (psum partitions
                    # 0:64 from enc_hi, 64:128 from enc_lo)
                    def enc_pair_ap(k):
                        return ep_tiles[k // 8][
                            :, (k % 8) * 2 * D_H : (k % 8 + 1) * 2 * D_H
                        ]
                elif encoder == "host":
                    enc_ap = load_enc_tiles(
                        enc_in, mm_dt, "enc", bitcast=f32r if main_f32r else None
                    )
                else:
                    w_sb = cpool.tile([D_IN, D_H], f32, tag="w")
                    nc.sync.dma_start(w_sb[:], w[:])
                    bias_sb = cpool.tile([128, D_H], f32, tag="bias")
                    nc.sync.dma_start(bias_sb[:], bias[:])
                    xt_sb = cpool.tile([D_IN, xt_cols], f32, tag="xt")
                    for j in range(0, xt_cols // 2048):
                        s = slice(j * 2048, (j + 1) * 2048)
                        nc.sync.dma_start(xt_sb[:, s], xt[:, s])

                    n_enc_chunks = xt_cols // 128
                    enc_parts = []
                    for k in range(n_enc_chunks):
                        pe = psenc.tile([128, D_H], f32, tag="psenc")
                        nc.tensor.matmul(
                            pe[:],
                            xt_sb[:, k * 128 : (k + 1) * 128],
                            w_sb[:],
                            start=True,
                            stop=True,
                        )
                        et = encpool.tile([128, D_H], mm_dt, tag=f"encp{k % 32}")
                        nc.vector.tensor_add(et[:], pe[:], bias_sb[:])
                        enc_parts.append(et)

                    if encoder == "replicated":
                        enc_ap = lambda k: enc_parts[k][:]  # noqa: E731
                    else:
                        # assemble shard in DRAM, AllGather, reload
                        shard_d = dram.tile([N_SH, D_H], f32, tag="shard")
                        for k in range(KC_SH):
                            nc.sync.dma_start(
                                shard_d[k * 128 : (k + 1) * 128, :], enc_parts[k][:]
                            )
                        full_d = dram.tile([N, D_H], f32, tag="full")
                        nc.gpsimd.collective_compute(
                            "AllGather",
                            mybir.AluOpType.bypass,
                            replica_groups=[list(range(N_CORES))],
                            ins=[shard_d.opt()],
                            outs=[full_d.opt()],
                        )
                        full_ap = full_d.opt()
                        enc_ap = load_enc_tiles(
                            full_ap.tensor,
                            mm_dt,
                            "enc",
                            src_offset=full_ap.offset,
                            bitcast=f32r if main_f32r else None,
                        )

                # ---- diffusion GEMM: outT[h, b] accumulated over 128 chunks.
                # rowsT streamed dma_g k-chunks per DMA (tile free index
                # g*B_LOC + b holds DRAM row g4*dma_g*128 + g*128 + p).
                out_ps = psout.tile(
                    [2 * D_H if pair else D_H, B_LOC], f32, tag="psout"
                )

                def rows_dma(handle, tag, g4):
                    rt = rpool.tile([128, dma_g * B_LOC], mm_dt, tag=tag)
                    src = bass.AP(
                        handle,
                        g4 * dma_g * 128 * B_LOC,
                        [[B_LOC, 128], [128 * B_LOC, dma_g], [1, B_LOC]],
                    )
                    if main_f32r:
                        src = src.bitcast(f32r)
                    nc.sync.dma_start(
                        rt[:].rearrange("p (g b) -> p g b", g=dma_g), src
                    )
                    return rt

                n_mm = 2 if pair else 1
                row_w = 2 * B_LOC if pair else B_LOC
                for g4 in range(KC // dma_g):
                    if pair:
                        rt = rpool.tile([128, dma_g * row_w], bf16, tag="rows")
                        src = bass.AP(
                            rows_pair,
                            g4 * dma_g * 128 * row_w,
                            [[row_w, 128], [128 * row_w, dma_g], [1, row_w]],
                        )
                        nc.sync.dma_start(
                            rt[:].rearrange("p (g b) -> p g b", g=dma_g), src
                        )
                    else:
                        rt = rows_dma(rowsT, "rows", g4)
                    for g in range(dma_g):
                        k = g4 * dma_g + g
                        bs = slice(g * row_w, g * row_w + B_LOC)
                        if pair:
                            bs_lo = slice(g * row_w + B_LOC, (g + 1) * row_w)
                            # one pass each of rows_hi and rows_lo against
                            # the combined [enc_hi | enc_lo] stationary:
                            # psum rows 0:64 accumulate enc_hi products,
                            # 64:128 accumulate enc_lo products (incl. the
                            # lo*lo term, a free accuracy bonus)
                            mms = [
                                (enc_pair_ap(k), rt[:, bs]),
                                (enc_pair_ap(k), rt[:, bs_lo]),
                            ]
                        else:
                            mms = [(enc_ap(k), rt[:, bs])]
                        for j, (lhs_ap, rhs_ap) in enumerate(mms):
                            nc.tensor.matmul(
                                out_ps[:],
                                lhs_ap,
                                rhs_ap,
                                start=(k == 0 and j == 0),
                                stop=(k == KC - 1 and j == n_mm - 1),
                            )

                outT_sb = opool.tile([D_H, B_LOC], f32, tag="res")
                if pair:
                    # DVE reads one PSUM operand max: copy hi half out, then
                    # add the lo half
                    nc.vector.tensor_copy(outT_sb[:], out_ps[0:D_H, :])
                    nc.vector.tensor_add(
                        outT_sb[:], outT_sb[:], out_ps[D_H : 2 * D_H, :]
                    )
                else:
                    nc.vector.tensor_copy(outT_sb[:], out_ps[:])
                nc.sync.dma_start(outT[:], outT_sb[:])

    nc.compile()
    return nc


def _split_bf16(x):
    import ml_dtypes

    hi = x.astype(ml_dtypes.bfloat16)
    lo = (x - hi.astype(np.float32)).astype(ml_dtypes.bfloat16)
    return hi, lo


def _pack_bf16_pair(x):
    """[n, m] fp32 -> [n, 2m] bf16 with hi in cols :m, lo in cols m:."""
    import ml_dtypes

    n, m = x.shape
    out = np.empty((n, 2 * m), dtype=ml_dtypes.bfloat16)
    out[:, :m] = x  # rounds to bf16 = hi
    out[:, m:] = x - out[:, :m].astype(np.float32)  # residual rounds = lo
    return out


def prepare_in_maps(X, ppr, W, b, idx, encoder="host", mm="fp32"):
    from concurrent.futures import ThreadPoolExecutor

    X = np.asarray(X, dtype=np.float32)
    ppr = np.asarray(ppr, dtype=np.float32)
    W = np.asarray(W, dtype=np.float32)
    b = np.asarray(b, dtype=np.float32)
    idx = np.asarray(idx).astype(np.int64)

    pair = mm == "bf16pair"

    def _rows_for_core(c):
        sel = idx[c * B_LOC : (c + 1) * B_LOC]
        rT = np.ascontiguousarray(ppr[sel].T)
        return _pack_bf16_pair(rT) if pair else rT

    with ThreadPoolExecutor(N_CORES) as ex:
        rowsT_per_core = list(ex.map(_rows_for_core, range(N_CORES)))

    if pair:
        enc = (X @ W + b).astype(np.float32)
        enc_pair = _pack_bf16_pair(enc)
        return [
            {"rows_pair": rowsT_per_core[c], "enc_pair": enc_pair}
            for c in range(N_CORES)
        ]

    if encoder == "host":
        enc = (X @ W + b).astype(np.float32)
        return [
            {"rowsT": rowsT_per_core[c], "enc": enc} for c in range(N_CORES)
        ]

    bias_bc = np.ascontiguousarray(np.broadcast_to(b, (128, D_H)))
    xt = np.ascontiguousarray(X.T)
    maps = []
    for c in range(N_CORES):
        if encoder == "replicated":
            xt_c = xt
        else:
            xt_c = np.ascontiguousarray(xt[:, c * N_SH : (c + 1) * N_SH])
        maps.append(
            {"rowsT": rowsT_per_core[c], "xt": xt_c, "w": W, "bias": bias_bc}
        )
    return maps


def _run_once(X, ppr, W, b, idx, encoder, mm):
    from concourse.bass_utils import run_bass_kernel_spmd

    global _compiled_nc, _compiled_mode
    if _compiled_nc is None or _compiled_mode != (encoder, mm):
        _compiled_nc = _build(encoder=encoder, mm=mm)
        _compiled_mode = (encoder, mm)
    nc = _compiled_nc

    in_maps = prepare_in_maps(X, ppr, W, b, idx, encoder=encoder, mm=mm)

    global _last_in_maps
    _last_in_maps = in_maps

    res = run_bass_kernel_spmd(nc, in_maps, list(range(N_CORES))).results
    out = np.concatenate([res[c]["outT"].T for c in range(N_CORES)], axis=0)
    return np.ascontiguousarray(out, dtype=np.float32)


def kernel(X, ppr, W, b, idx, encoder="host", mm="bf16pair"):
    import time

    if mm == "bf16pair":
        try:
            import ml_dtypes  # noqa: F401
        except ImportError:
            mm = "fp32"  # same kernel at fp32-native precision, ~10% slower

    # The shared trn2 devices occasionally throw transient errors
    # (NRT_EXEC_UNIT_UNRECOVERABLE / mesh desynced); retry before giving up.
    last_exc = None
    for attempt in range(3):
        try:
            return _run_once(X, ppr, W, b, idx, encoder, mm)
        except Exception as e:  # noqa: BLE001
            last_exc = e
            global _compiled_nc, _compiled_mode
            _compiled_nc = None
            _compiled_mode = None
            time.sleep(5 * (attempt + 1))
    raise last_exc


# revision 44
# speedup vs baseline: 1.1439x; 1.1439x over previous
"""DenseNibblePPR diffusion kernel for 8 Trainium2 NeuronCores.

Math: out = ppr[idx] @ (X @ W + b),  shapes:
  X [16384, 128] f32, ppr [16384, 16384] f32, W [128, 64] f32,
  b [64] f32, idx [4096] i64  ->  out [4096, 64] f32.

Sharding strategy (batch / seed-node parallel, deduplicated):
  idx samples seeds WITH REPLACEMENT, so only ~3650 of the 4096
  gathered PPR rows are distinct. The device processes the unique
  rows (464 slots/core, zero-padded; host replicates duplicate
  outputs via the inverse map afterward), cutting the dominant HBM
  stream ~11% below the naive roofline. Inputs whose unique count
  exceeds 8*464 fall back to the dense 512-slot path. Otherwise: the
  unique seeds are split across the 8 cores (≤464 each). Each
  core receives its 512 gathered PPR rows, pre-transposed to
  [16384, 512] so the contraction dim (nodes) lands on SBUF
  partitions, plus the full [16384, 64] encoder table enc = X @ W + b
  (the encoder is 3% of the FLOPs; it is evaluated once during input
  sharding rather than redundantly per core). Each core streams its
  33.5 MB row shard from HBM in 1 MiB grouped DMAs and accumulates
  outT[64, 512] over 128 k-chunks in a single PSUM fp32 accumulation
  chain on the tensor engine. The host concatenates the per-core
  [512, 64] results. No collectives.

  GEMM precision (mm="bf16pair", default): fp32 operands are split
  into bf16 hi+lo pairs (packed hi|lo along the free dim so the DMA
  shape matches the fp32 layout exactly — same total bytes). The
  diffusion matmul only needs 64 stationary columns, so [enc_hi |
  enc_lo] loads as one [128, 128] stationary and each k-chunk takes
  just 2 matmul passes (rows_hi, rows_lo): PSUM partitions 0:64
  accumulate the enc_hi products and 64:128 the enc_lo products
  (including the lo*lo term), summed once by DVE at the end. This is
  bf16x4-grade fp32 emulation: measured end-to-end error 3.9e-6 (vs
  5.9e-7 for native fp32 matmuls) with the PE at 1 cycle/row instead
  of fp32's 4, which moves the kernel from PE-bound (114 us/core) to
  the measured HBM roofline (~101-104 us/core for the 37.9 MB/core
  stream, ~370 GB/s/core).

  Alternatives kept behind flags, all verified correct on HW:
    mm="fp32"  native fp32 matmuls, err 5.9e-7, ~114 us (PE-bound)
    mm="f32r"  TRN2 reduced-precision fp32 mode, err 1.5e-4, ~103 us
    encoder="replicated"/"allgather": on-device encoder variants;
    slower (131 us / ~180 us) — redundant encoder work or the
    AllGather sit on the PE critical path.
"""

import numpy as np

N = 16384
D_IN = 128
D_H = 64
B = 4096
N_CORES = 8
B_LOC = B // N_CORES  # 512
KC = N // 128  # 128 contraction chunks of 128 nodes
N_SH = N // N_CORES  # 2048 encoder shard rows per core
KC_SH = N_SH // 128  # 16 encoder chunks per core

_compiled_nc = None
_compiled_mode = None
_last_in_maps = None


def _build(reps=1, encoder="host", mm="fp32", dma_g=4, rows_bufs=8, main_f32r=None, b_loc=B_LOC):
    import concourse.bacc as bacc
    import concourse.bass as bass
    import concourse.mybir as mybir
    import concourse.tile as tile

    if main_f32r:  # legacy alias
        mm = "f32r"
    f32 = mybir.dt.float32
    f32r = mybir.dt.float32r
    bf16 = mybir.dt.bfloat16
    main_f32r = mm == "f32r"
    pair = mm == "bf16pair"
    assert not (pair and encoder != "host"), "bf16pair requires host encoder"
    mm_dt = {"fp32": f32, "f32r": f32r, "bf16pair": bf16}[mm]

    nc = bacc.Bacc("TRN2", target_bir_lowering=False, debug=False, num_devices=N_CORES)

    if pair:
        # hi|lo planes packed along the free dim: row n = [hi(512|64), lo(...)]
        rows_pair = nc.dram_tensor("rows_pair", [N, 2 * b_loc], bf16, kind="ExternalInput")
        enc_pair = nc.dram_tensor("enc_pair", [N, 2 * D_H], bf16, kind="ExternalInput")
    elif encoder == "host":
        rowsT = nc.dram_tensor("rowsT", [N, b_loc], f32, kind="ExternalInput")
        enc_in = nc.dram_tensor("enc", [N, D_H], f32, kind="ExternalInput")
    else:
        rowsT = nc.dram_tensor("rowsT", [N, b_loc], f32, kind="ExternalInput")
        xt_cols = N if encoder == "replicated" else N_SH
        xt = nc.dram_tensor("xt", [D_IN, xt_cols], f32, kind="ExternalInput")
        w = nc.dram_tensor("w", [D_IN, D_H], f32, kind="ExternalInput")
        bias = nc.dram_tensor("bias", [128, D_H], f32, kind="ExternalInput")
    outT = nc.dram_tensor("outT", [D_H, b_loc], f32, kind="ExternalOutput")

    with tile.TileContext(nc) as tc:
        with (
            tc.tile_pool(name="const", bufs=1) as cpool,
            tc.tile_pool(name="enc", bufs=2 if encoder == "replicated" else 1) as encpool,
            tc.tile_pool(name="rows", bufs=rows_bufs) as rpool,
            tc.tile_pool(name="res", bufs=2) as opool,
            tc.tile_pool(name="psenc", bufs=4, space="PSUM") as psenc,
            tc.tile_pool(name="psout", bufs=2, space="PSUM") as psout,
            tc.tile_pool(name="dram", bufs=1, space="DRAM") as dram,
        ):
            for _rep in range(reps):
                # ---- encoder table: enc[n, h], n on partitions, 128 chunks
                # stored as 16 SBUF tiles [128, 8*64] (8 chunks each)
                def load_enc_tiles(src_handle, dtype, tagp, src_offset=0, bitcast=None):
                    tiles = []
                    for j in range(16):
                        t = encpool.tile([128, 8 * D_H], dtype, tag=f"{tagp}{j}")
                        src = bass.AP(
                            src_handle,
                            src_offset + j * 1024 * D_H,
                            [[D_H, 128], [128 * D_H, 8], [1, D_H]],
                        )
                        if bitcast is not None:
                            src = src.bitcast(bitcast)
                        nc.sync.dma_start(
                            t[:].rearrange("p (g h) -> p g h", g=8), src
                        )
                        tiles.append(t)
                    return lambda k: tiles[k // 8][
                        :, (k % 8) * D_H : (k % 8 + 1) * D_H
                    ]

                if pair:
                    ep_tiles = []
                    for j in range(16):
                        t = encpool.tile([128, 8 * 2 * D_H], bf16, tag=f"enc{j}")
                        src = bass.AP(
                            enc_pair,
                            j * 1024 * 2 * D_H,
                            [[2 * D_H, 128], [128 * 2 * D_H, 8], [1, 2 * D_H]],
                        )
                        nc.sync.dma_start(
                            t[:].rearrange("p (g h) -> p g h", g=8), src
                        )
                        ep_tiles.append(t)

                    # [enc_hi | enc_lo] as one [128, 128] stationary: one
                    # matmul pass produces both products (psum partitions
                    # 0:64 from enc_hi, 64:128 from enc_lo)
                    def enc_pair_ap(k):
                        return ep_tiles[k // 8][
                            :, (k % 8) * 2 * D_H : (k % 8 + 1) * 2 * D_H
                        ]
                elif encoder == "host":
                    enc_ap = load_enc_tiles(
                        enc_in, mm_dt, "enc", bitcast=f32r if main_f32r else None
                    )
                else:
                    w_sb = cpool.tile([D_IN, D_H], f32, tag="w")
                    nc.sync.dma_start(w_sb[:], w[:])
                    bias_sb = cpool.tile([128, D_H], f32, tag="bias")
                    nc.sync.dma_start(bias_sb[:], bias[:])
                    xt_sb = cpool.tile([D_IN, xt_cols], f32, tag="xt")
                    for j in range(0, xt_cols // 2048):
                        s = slice(j * 2048, (j + 1) * 2048)
                        nc.sync.dma_start(xt_sb[:, s], xt[:, s])

                    n_enc_chunks = xt_cols // 128
                    enc_parts = []
                    for k in range(n_enc_chunks):
                        pe = psenc.tile([128, D_H], f32, tag="psenc")
                        nc.tensor.matmul(
                            pe[:],
                            xt_sb[:, k * 128 : (k + 1) * 128],
                            w_sb[:],
                            start=True,
                            stop=True,
                        )
                        et = encpool.tile([128, D_H], mm_dt, tag=f"encp{k % 32}")
                        nc.vector.tensor_add(et[:], pe[:], bias_sb[:])
                        enc_parts.append(et)

                    if encoder == "replicated":
                        enc_ap = lambda k: enc_parts[k][:]  # noqa: E731
                    else:
                        # assemble shard in DRAM, AllGather, reload
                        shard_d = dram.tile([N_SH, D_H], f32, tag="shard")
                        for k in range(KC_SH):
                            nc.sync.dma_start(
                                shard_d[k * 128 : (k + 1) * 128, :], enc_parts[k][:]
                            )
                        full_d = dram.tile([N, D_H], f32, tag="full")
                        nc.gpsimd.collective_compute(
                            "AllGather",
                            mybir.AluOpType.bypass,
                            replica_groups=[list(range(N_CORES))],
                            ins=[shard_d.opt()],
                            outs=[full_d.opt()],
                        )
                        full_ap = full_d.opt()
                        enc_ap = load_enc_tiles(
                            full_ap.tensor,
                            mm_dt,
                            "enc",
                            src_offset=full_ap.offset,
                            bitcast=f32r if main_f32r else None,
                        )

                # ---- diffusion GEMM: outT[h, b] accumulated over 128 chunks.
                # rowsT streamed dma_g k-chunks per DMA (tile free index
                # g*b_loc + b holds DRAM row g4*dma_g*128 + g*128 + p).
                out_ps = psout.tile(
                    [2 * D_H if pair else D_H, b_loc], f32, tag="psout"
                )

                def rows_dma(handle, tag, g4):
                    rt = rpool.tile([128, dma_g * b_loc], mm_dt, tag=tag)
                    src = bass.AP(
                        handle,
                        g4 * dma_g * 128 * b_loc,
                        [[b_loc, 128], [128 * b_loc, dma_g], [1, b_loc]],
                    )
                    if main_f32r:
                        src = src.bitcast(f32r)
                    nc.sync.dma_start(
                        rt[:].rearrange("p (g b) -> p g b", g=dma_g), src
                    )
                    return rt

                n_mm = 2 if pair else 1
                row_w = 2 * b_loc if pair else b_loc
                for g4 in range(KC // dma_g):
                    if pair:
                        rt = rpool.tile([128, dma_g * row_w], bf16, tag="rows")
                        src = bass.AP(
                            rows_pair,
                            g4 * dma_g * 128 * row_w,
                            [[row_w, 128], [128 * row_w, dma_g], [1, row_w]],
                        )
                        nc.sync.dma_start(
                            rt[:].rearrange("p (g b) -> p g b", g=dma_g), src
                        )
                    else:
                        rt = rows_dma(rowsT, "rows", g4)
                    for g in range(dma_g):
                        k = g4 * dma_g + g
                        bs = slice(g * row_w, g * row_w + b_loc)
                        if pair:
                            bs_lo = slice(g * row_w + b_loc, (g + 1) * row_w)
                            # one pass each of rows_hi and rows_lo against
                            # the combined [enc_hi | enc_lo] stationary:
                            # psum rows 0:64 accumulate enc_hi products,
                            # 64:128 accumulate enc_lo products (incl. the
                            # lo*lo term, a free accuracy bonus)
                            mms = [
                                (enc_pair_ap(k), rt[:, bs]),
                                (enc_pair_ap(k), rt[:, bs_lo]),
                            ]
                        else:
                            mms = [(enc_ap(k), rt[:, bs])]
                        for j, (lhs_ap, rhs_ap) in enumerate(mms):
                            nc.tensor.matmul(
                                out_ps[:],
                                lhs_ap,
                                rhs_ap,
                                start=(k == 0 and j == 0),
                                stop=(k == KC - 1 and j == n_mm - 1),
                            )

                outT_sb = opool.tile([D_H, b_loc], f32, tag="res")
                if pair:
                    # DVE reads one PSUM operand max: copy hi half out, then
                    # add the lo half
                    nc.vector.tensor_copy(outT_sb[:], out_ps[0:D_H, :])
                    nc.vector.tensor_add(
                        outT_sb[:], outT_sb[:], out_ps[D_H : 2 * D_H, :]
                    )
                else:
                    nc.vector.tensor_copy(outT_sb[:], out_ps[:])
                nc.sync.dma_start(outT[:], outT_sb[:])

    nc.compile()
    return nc


def _split_bf16(x):
    import ml_dtypes

    hi = x.astype(ml_dtypes.bfloat16)
    lo = (x - hi.astype(np.float32)).astype(ml_dtypes.bfloat16)
    return hi, lo


def _pack_bf16_pair(x):
    """[n, m] fp32 -> [n, 2m] bf16 with hi in cols :m, lo in cols m:."""
    import ml_dtypes

    n, m = x.shape
    out = np.empty((n, 2 * m), dtype=ml_dtypes.bfloat16)
    out[:, :m] = x  # rounds to bf16 = hi
    out[:, m:] = x - out[:, :m].astype(np.float32)  # residual rounds = lo
    return out


def prepare_in_maps(X, ppr, W, b, idx, encoder="host", mm="fp32", sels=None):
    from concurrent.futures import ThreadPoolExecutor

    X = np.asarray(X, dtype=np.float32)
    ppr = np.asarray(ppr, dtype=np.float32)
    W = np.asarray(W, dtype=np.float32)
    b = np.asarray(b, dtype=np.float32)
    idx = np.asarray(idx).astype(np.int64)

    pair = mm == "bf16pair"
    if sels is None:
        sels = [idx[c * B_LOC : (c + 1) * B_LOC] for c in range(N_CORES)]

    def _rows_for_core(c):
        rT = np.ascontiguousarray(ppr[sels[c]].T)
        return _pack_bf16_pair(rT) if pair else rT

    with ThreadPoolExecutor(N_CORES) as ex:
        rowsT_per_core = list(ex.map(_rows_for_core, range(N_CORES)))

    if pair:
        enc = (X @ W + b).astype(np.float32)
        enc_pair = _pack_bf16_pair(enc)
        return [
            {"rows_pair": rowsT_per_core[c], "enc_pair": enc_pair}
            for c in range(N_CORES)
        ]

    if encoder == "host":
        enc = (X @ W + b).astype(np.float32)
        return [
            {"rowsT": rowsT_per_core[c], "enc": enc} for c in range(N_CORES)
        ]

    bias_bc = np.ascontiguousarray(np.broadcast_to(b, (128, D_H)))
    xt = np.ascontiguousarray(X.T)
    maps = []
    for c in range(N_CORES):
        if encoder == "replicated":
            xt_c = xt
        else:
            xt_c = np.ascontiguousarray(xt[:, c * N_SH : (c + 1) * N_SH])
        maps.append(
            {"rowsT": rowsT_per_core[c], "xt": xt_c, "w": W, "bias": bias_bc}
        )
    return maps


B_U = 464  # per-core slots on the deduplicated path (8*464 = 3712 >= +4 sigma of E[unique]=3624)


def _run_once(X, ppr, W, b, idx, encoder, mm):
    from concourse.bass_utils import run_bass_kernel_spmd

    # idx samples seeds WITH REPLACEMENT (~11% duplicate rows); the device
    # only needs the unique rows — outputs for duplicates are replicated on
    # the host via the inverse map. Falls back to the dense path when the
    # unique count exceeds capacity.
    idx_arr = np.asarray(idx).astype(np.int64)
    uniq, inv = np.unique(idx_arr, return_inverse=True)
    dedup = len(uniq) <= N_CORES * B_U
    b_loc = B_U if dedup else B_LOC
    if dedup:
        sel_flat = np.concatenate(
            [uniq, np.zeros(N_CORES * B_U - len(uniq), dtype=np.int64)]
        )
        sels = [sel_flat[c * B_U : (c + 1) * B_U] for c in range(N_CORES)]
    else:
        sels = None

    global _compiled_nc, _compiled_mode
    if _compiled_nc is None or _compiled_mode != (encoder, mm, b_loc):
        _compiled_nc = _build(encoder=encoder, mm=mm, b_loc=b_loc)
        _compiled_mode = (encoder, mm, b_loc)
    nc = _compiled_nc

    in_maps = prepare_in_maps(X, ppr, W, b, idx_arr, encoder=encoder, mm=mm, sels=sels)

    global _last_in_maps
    _last_in_maps = in_maps

    res = run_bass_kernel_spmd(nc, in_maps, list(range(N_CORES))).results
    out = np.concatenate([res[c]["outT"].T for c in range(N_CORES)], axis=0)
    if dedup:
        out = out[inv]
    return np.ascontiguousarray(out, dtype=np.float32)


def kernel(X, ppr, W, b, idx, encoder="host", mm="bf16pair"):
    import time

    if mm == "bf16pair":
        try:
            import ml_dtypes  # noqa: F401
        except ImportError:
            mm = "fp32"  # same kernel at fp32-native precision, ~10% slower

    # The shared trn2 devices occasionally throw transient errors
    # (NRT_EXEC_UNIT_UNRECOVERABLE / mesh desynced); retry before giving up.
    last_exc = None
    for attempt in range(3):
        try:
            return _run_once(X, ppr, W, b, idx, encoder, mm)
        except Exception as e:  # noqa: BLE001
            last_exc = e
            global _compiled_nc, _compiled_mode
            _compiled_nc = None
            _compiled_mode = None
            time.sleep(5 * (attempt + 1))
    raise last_exc
